# revision 40
# baseline (speedup 1.0000x reference)
"""TRN2 Bass kernel for nn_Attender:
    weights[b, s] = sum_d (state @ W.T + bias)[b, d] * enc[s, b, d]
with enc [S=2048, B=16, D=2048], state [B, D], W [D, D], bias [D], out [B, S].

Sharding (8 NeuronCores): the contraction dim D is split into 8 slices of 256,
one per core. Each core computes alteredT[d_k, b] = (W[d_k, :] @ state.T +
bias[d_k]) — needing only a 256-row slice of W — and the partial score
partial_k[b, s] = sum_{d in d_k} altered[b, d] * enc[s, b, d]. The host sums
the 8 partials (a pure reduction un-shard); no cross-device communication.

Device layout (host-pretransposed so every DMA is partition-contiguous):
  enc  [2, 128, 16*F]  per-core enc slice with d on partitions:
                       [chunk c, partition p, (batch b, free)]
                       free = s (fp16x1) or (hi/lo, s) (bf16x3)
  wp   [128, 16*256]   wp[p, i*256+d] = W[k*256+d, i*128+p]   (lhsT tiles)
  sp   [128, 16*16]    sp[p, i*16+b]  = state[b, i*128+p]
  bk   [128, 2]        bk[p, c]       = bias[k*256 + c*128 + p]  (fp32)

Main contraction on the PE: lhsT = alteredT[d_chunk, b] (M=1), rhs =
encT[d_chunk, s] (N=512), accumulating d-chunks (and hi/lo passes) in PSUM.
The 4 batches of a group land in one PSUM bank at partitions {0,32,64,96}
via explicit tile_position col-tiling, so a single [128, 512] copy (DVE/ACT
alternating) drains 4 batches, and one strided DMA per group ships [4, S].

enc streams as variable-size tilesets (8/4/2/1/1 batches for fp16x1): big
transfers early for DMA efficiency, small at the end so the post-stream
compute tail is one batch (~2us). In fp16x1 every tileset owns its SBUF slot
— the stream never waits on compute; bf16x3 tiles are 2x as large and use
4-batch per-chunk tiles with 2 rotating slots instead. Constants ride the
SWDGE (gpsimd) queue so the sync HWDGE queue streams enc from its first
instruction.

Precision modes (BASS_KERNEL_MODE env, default fp16x1):
  fp16x1: enc/W/state/altered in fp16, one product pass, fp32 PSUM accumulate.
          Halves HBM traffic (the kernel is DMA-bound). Measured error:
          max|err| = 3.7e-4 * max|ref| = 1.5e-3 * rms(ref); resid_var 1.6e-7.
          Matches an exact numpy fp16 simulation — pure input-rounding error.
  bf16x3: enc and altered split into bf16 hi+lo on the host/device; 3
          accumulated products (hi*hi + lo*hi + hi*lo) recover ~17 mantissa
          bits -> max|err| = 1.9e-5 * rms(ref), at 2x the HBM traffic.
"""

import os
from contextlib import ExitStack

import numpy as np
import ml_dtypes

import concourse.bacc as bacc
import concourse.tile as tile
import concourse.mybir as mybir
from concourse.bass_utils import run_bass_kernel_spmd

S, B, D = 2048, 16, 2048
NCORES = 8
DK = D // NCORES  # 256 contraction elems per core
NCH = DK // 128  # 2 partition chunks
BG = 4  # batches per psum group
NG = B // BG  # 4 groups
ST = 512  # s-tile (one PSUM bank)
NST = S // ST  # 4 s-tiles

MODE = os.environ.get("BASS_KERNEL_MODE", "fp16x1")

F32 = mybir.dt.float32
F16 = mybir.dt.float16
BF16 = mybir.dt.bfloat16

_CACHE = {}

LAST_RESULTS = None


def _build(mode):
    nc = bacc.Bacc("TRN2", target_bir_lowering=False, debug=False, num_devices=NCORES)

    if mode == "bf16x3":
        efree, edt, wdt = 2 * S, BF16, F32
    else:
        efree, edt, wdt = S, F16, F16
    # Enc tilesets in HALF-BATCH units: big transfers early (DMA efficiency),
    # a half-batch last so the post-stream compute tail is ~4 matmuls. bf16x3
    # tiles are 2x as large and keep whole-batch tilesets with rotating slots.
    HS = efree // 2  # free elems per half-batch
    if mode == "bf16x3":
        TS_SIZES = [4, 4, 4, 4]  # whole batches
        ts_start = [sum(TS_SIZES[:i]) for i in range(len(TS_SIZES))]
        b2ts = {}
        for t, (sz, st0) in enumerate(zip(TS_SIZES, ts_start)):
            for j in range(sz):
                b2ts[st0 + j] = (t, j)
    else:
        TS_SIZES = [16, 8, 4, 2, 1, 1]  # half-batches
        assert sum(TS_SIZES) == 2 * B
        ts_start = [sum(TS_SIZES[:i]) for i in range(len(TS_SIZES))]
        hb2ts = {}
        for t, (sz, st0) in enumerate(zip(TS_SIZES, ts_start)):
            for j in range(sz):
                hb2ts[st0 + j] = (t, j)
    ENC = nc.dram_tensor(
        "enc", [NCH, 128, B * efree], edt, kind="ExternalInput"
    ).ap()
    WP = nc.dram_tensor("wp", [128, 16 * DK], wdt, kind="ExternalInput").ap()
    SP = nc.dram_tensor("sp", [128, 16 * B], wdt, kind="ExternalInput").ap()
    BK = nc.dram_tensor("bk", [128, NCH], F32, kind="ExternalInput").ap()
    OUT = nc.dram_tensor("out", [B, S], F32, kind="ExternalOutput").ap()

    with tile.TileContext(nc) as tc, ExitStack() as ctx:
        cpool = ctx.enter_context(tc.tile_pool(name="const", bufs=1))
        epool = ctx.enter_context(tc.tile_pool(name="enc", bufs=1))
        opool = ctx.enter_context(tc.tile_pool(name="outp", bufs=2))
        apsum = ctx.enter_context(tc.tile_pool(name="apsum", bufs=2, space="PSUM"))
        mpsum = ctx.enter_context(tc.tile_pool(name="mpsum", bufs=6, space="PSUM"))

        # Constants ride the SWDGE (gpsimd) path so the HWDGE (sync) queue
        # streams enc tiles from instruction 0.
        wp_t = cpool.tile([128, 16 * DK], wdt, tag="wp")
        nc.gpsimd.dma_start(wp_t[:], WP[:])
        sp_t = cpool.tile([128, 16 * B], wdt, tag="sp")
        nc.gpsimd.dma_start(sp_t[:], SP[:])
        bk_t = cpool.tile([128, NCH], F32, tag="bk")
        nc.gpsimd.dma_start(bk_t[:], BK[:])

        # alteredT[d, b] = sum_i W[d, i] * state[b, i] + bias[d], d on partitions.
        amats = []  # amats[c] = lhsT tiles, one per product pass
        for c in range(NCH):
            aps = apsum.tile([128, B], F32, tag="aps")
            for i in range(16):
                nc.tensor.matmul(
                    aps[:],
                    wp_t[:, i * DK + c * 128 : i * DK + (c + 1) * 128],
                    sp_t[:, i * B : (i + 1) * B],
                    start=(i == 0),
                    stop=(i == 15),
                )
            altf = cpool.tile([128, B], F32, tag=f"altf{c}")
            nc.vector.tensor_scalar_add(altf[:], aps[:], bk_t[:, c : c + 1])
            if mode == "bf16x3":
                ahi = cpool.tile([128, B], BF16, tag=f"ahi{c}")
                nc.vector.tensor_copy(ahi[:], altf[:])
                ahif = cpool.tile([128, B], F32, tag=f"ahif{c}")
                nc.vector.tensor_copy(ahif[:], ahi[:])
                alof = cpool.tile([128, B], F32, tag=f"alof{c}")
                nc.vector.tensor_sub(alof[:], altf[:], ahif[:])
                alo = cpool.tile([128, B], BF16, tag=f"alo{c}")
                nc.vector.tensor_copy(alo[:], alof[:])
                amats.append([ahi, alo])
            else:
                af = cpool.tile([128, B], F16, tag=f"af{c}")
                nc.vector.tensor_copy(af[:], altf[:])
                amats.append([af])

        # passes: (a-tile index, enc hi/lo offset)
        if mode == "bf16x3":
            passes = [(0, 0), (1, 0), (0, 1)]
        else:
            passes = [(0, 0)]
        n_mm = len(passes) * NCH

        # Every tileset gets its own SBUF slot, so the enc stream never waits
        # on compute (~128KB/partition total). Both d-chunks ride one DMA:
        # tile free layout is (c, b_local, s).
        tsets = {}  # t -> tile (fp16x1) or [tile per chunk] (bf16x3)
        if mode == "bf16x3":
            # 2x-size tiles exceed SBUF with own-slot tilesets; use per-chunk
            # DMAs with 2 rotating slots per chunk tag (PE-heavy mode anyway).
            for t, (sz, st0) in enumerate(zip(TS_SIZES, ts_start)):
                tiles_c = []
                for c in range(NCH):
                    et = epool.tile(
                        [128, sz * efree],
                        edt,
                        tag=f"enc{c}",
                        bufs=2,
                        name=f"e_{t}_{c}",
                    )
                    nc.sync.dma_start(
                        et[:], ENC[c, :, st0 * efree : (st0 + sz) * efree]
                    )
                    tiles_c.append(et)
                tsets[t] = tiles_c
        else:
            for t, (sz, st0) in enumerate(zip(TS_SIZES, ts_start)):
                et = epool.tile(
                    [128, NCH * sz * HS],
                    edt,
                    tag=f"enct{t}",
                    name=f"e_{t}",
                )
                nc.sync.dma_start(
                    et[:].rearrange("p (c f) -> p c f", c=NCH),
                    ENC[:, :, st0 * HS : (st0 + sz) * HS].rearrange(
                        "c p f -> p c f"
                    ),
                )
                tsets[t] = et

        out_r = OUT.rearrange("(g bi) s -> g bi s", bi=BG)
        for g in range(NG):
            pts = [
                mpsum.tile([128, ST], F32, tag="mm", name=f"pt_{g}_{st}")
                for st in range(NST)
            ]
            for bi in range(BG):
                b = g * BG + bi
                for st in range(NST):
                    if mode == "bf16x3":
                        t, bloc = b2ts[b]
                    else:
                        # half-batch hb = (b, s-half); st 0-1 in the first
                        # half, st 2-3 in the second
                        t, hbloc = hb2ts[2 * b + (1 if st >= 2 else 0)]
                    sz = TS_SIZES[t]
                    k = 0
                    for aj, hl in passes:
                        for c in range(NCH):
                            if mode == "bf16x3":
                                rhs_t = tsets[t][c]
                                off = bloc * efree + hl * S + st * ST
                            else:
                                rhs_t = tsets[t]
                                off = (c * sz + hbloc) * HS + (st % 2) * ST
                            nc.tensor.matmul(
                                pts[st][32 * bi : 32 * bi + 1, :],
                                amats[c][aj][:, b : b + 1],
                                rhs_t[:, off : off + ST],
                                start=(k == 0),
                                stop=(k == n_mm - 1),
                                tile_position=(0, 32 * bi),
                            )
                            k += 1
            # Stage the group's [4, S] result (batch bi at partition 32*bi)
            # and ship it while later groups stream. Output DMAs ride the ACT
            # HWDGE queue so they never block enc streaming on the sync queue.
            outg = opool.tile([128, S], F32, tag="outg", name=f"outg_{g}")
            for st in range(NST):
                dst = outg[:, st * ST : (st + 1) * ST]
                if st % 2 == 0:
                    nc.vector.tensor_copy(dst, pts[st][:])
                else:
                    nc.scalar.copy(dst, pts[st][:])
            src_r = outg[:].rearrange("(bi r) s -> bi r s", bi=BG)[:, 0]
            nc.scalar.dma_start(out_r[g], src_r)

    nc.compile()
    return nc


def _prep_inputs(encoder_outputs, state, W, b, mode):
    """Build the 8 per-core input maps (heavy layout work on host)."""
    f32 = np.float32
    bf16 = ml_dtypes.bfloat16
    wnp = f32 if mode == "bf16x3" else np.float16
    in_maps = []
    # [S, B, D] -> [B, D, S] once
    encT = np.ascontiguousarray(encoder_outputs.transpose(1, 2, 0))
    spk = np.ascontiguousarray(
        state.T.reshape(16, 128, B).transpose(1, 0, 2).reshape(128, 16 * B)
    ).astype(wnp)
    for k in range(NCORES):
        d0 = k * DK
        e = encT[:, d0 : d0 + DK, :]  # [B, DK, S]
        # -> [c, p, B, S]
        e = np.ascontiguousarray(e.reshape(B, NCH, 128, S).transpose(1, 2, 0, 3))
        if mode == "bf16x3":
            ehi = e.astype(bf16)
            elo = (e - ehi.astype(f32)).astype(bf16)
            # [c, p, B, S] -> [c, p, b, hl, s] -> [c, p, B*2S]
            enc_k = np.ascontiguousarray(
                np.stack([ehi, elo], axis=3).reshape(NCH, 128, B * 2 * S)
            )
        else:
            enc_k = np.ascontiguousarray(
                e.astype(np.float16).reshape(NCH, 128, B * S)
            )
        wp = np.ascontiguousarray(
            W[d0 : d0 + DK, :].T.reshape(16, 128, DK).transpose(1, 0, 2).reshape(128, 16 * DK)
        ).astype(wnp)
        bk = np.ascontiguousarray(b[d0 : d0 + DK].reshape(NCH, 128).T)
        in_maps.append({"enc": enc_k, "wp": wp, "sp": spk, "bk": bk})
    return in_maps


def kernel(encoder_outputs, state, W, b):
    global LAST_RESULTS
    mode = MODE
    if mode not in _CACHE:
        _CACHE[mode] = _build(mode)
    nc = _CACHE[mode]
    in_maps = _prep_inputs(
        np.asarray(encoder_outputs, dtype=np.float32),
        np.asarray(state, dtype=np.float32),
        np.asarray(W, dtype=np.float32),
        np.asarray(b, dtype=np.float32),
        mode,
    )
    res = run_bass_kernel_spmd(nc, in_maps, core_ids=list(range(NCORES)))
    LAST_RESULTS = res
    acc = np.zeros((B, S), dtype=np.float64)
    for k in range(NCORES):
        acc += res.results[k]["out"].astype(np.float64)
    return acc.astype(np.float32)
